# revision 33
# baseline (speedup 1.0000x reference)
"""Trainium2 Bass kernel for LittleBitLinear reconstruction (fp8 DoubleRow).

Computes M = (sign(U_fp) * ell) @ sign(V_fp)^T * g[None, :] * h[:, None]
for U_fp (4096, 1024), V_fp (11008, 1024) -> M (4096, 11008) fp32.

Strategy: shard d_in (rows of V_fp / columns of M) across 8 cores; U_fp, h,
ell replicated. Each core computes the full 4096 x 1376 column block.

The contraction-dim scale ell is split per-r into a product of two fp8-e4m3
grid values ell_r ~= a_r * b_r (pair-optimized on host over the e4m3 grid, a
few % the error of a single fp8 round). Host input prep quantizes the sign
operands straight to fp8 (exact: +-a_r / +-b_r are grid values):
  A[r, o] = sign(U^T)[r, o] * a_r          (fp8)
  B[r, i] = sign(V^T)[r, i] * b_r          (fp8; sign(ell) folded in b)
so the device runs a pure matmul stream at the measured fp8-DoubleRow
roofline (1 col/cycle @ 2.4 GHz with K=256 per pass, 157 TF/s/core; 73.4 us
of PE work per core):
  psum    = A-slice^T @ B                  (DoubleRow, fp32 PSUM)
  out     = bf16(psum * h_o) * bf16(g_i)   (evac + DVE mult, bf16 store)
Host upcasts bf16 -> fp32. Measured end-to-end absmax rel err ~7.8e-3.

Layout/schedule notes (from perfetto traces):
- A/B ship partition-major [128, KB, cols] so one DMA covers all 8 k-planes
  of a column range (1024 descriptors instead of 8 separate plane DMAs).
- The prologue is HBM-bandwidth bound (~300 GB/s/core aggregate, fair-shared
  across in-flight transfers), so B halves ([0:512] first — they feed
  ntile 0) alternate sync/scalar, the narrow A chunk 0 rides scalar, and
  later A chunks are paced: chunk 1 is gated behind a dummy DVE op that
  reads the last B half (WAW on its tile), chunks 2+ WAR-pace through the
  bufs=2 pool recycle. Unpaced, the A transfers starve B and stall the PE.
- Block 0 runs ntile-outer (its first matmuls need only B's [0:512] halves)
  with no-dep dummy matmuls between groups to keep the PE p-state ramping
  through the fill; blocks 1+ run dstep-outer.
- Blocks 0-1 evacuate on DVE tensor_scalar straight from PSUM: the tile
  scheduler orders waits by its cost-model sim (which models fp8-DR 2x
  faster than real HW), handing the early ACT evacs a ~10-matmul-late wait
  that stalls the block-2 PSUM recycle.
- The last block's evac/store splits in halves across both HWDGE queues to
  shrink the post-stream tail.
"""

import os
import sys

import numpy as np

for _p in ("/opt/trn_rl_repo",):
    if _p not in sys.path and os.path.isdir(_p):
        sys.path.insert(0, _p)

D_OUT, D_IN, R, NCORES = 4096, 11008, 1024, 8
N_SH = D_IN // NCORES  # 1376
P = 128
KB = R // P            # 8 k-planes
NTILES = [(0, 512), (512, 512), (1024, 352)]
BSPLIT = 512           # B plane-pair DMA halves: [0:512], [512:1376]
# A chunks (d_out col ranges): narrow first chunk so block 0 unblocks fast
CHUNKS = [(0, 256), (256, 768), (1024, 1024), (2048, 1024), (3072, 1024)]
N_WARMUP = 4


def build_program():
    """Build the per-core Bass program (SPMD: same program, different data)."""
    from contextlib import ExitStack

    import concourse.bass as bass  # noqa: F401
    import concourse.mybir as mybir
    import concourse.tile as tile
    from concourse import bacc

    f32 = mybir.dt.float32
    bf16 = mybir.dt.bfloat16
    fp8 = mybir.dt.float8e4
    AF = mybir.ActivationFunctionType
    ALU = mybir.AluOpType
    DR = mybir.MatmulPerfMode.DoubleRow

    oblocks = D_OUT // P   # 32

    nc = bacc.Bacc(None, target_bir_lowering=False)
    # partition-major 3D layouts so one DMA covers all 8 k-planes of a slice
    at_d = nc.declare_dram_parameter("at", [P, KB, D_OUT], fp8, isOutput=False)
    bt_d = nc.declare_dram_parameter("bt", [P, KB, N_SH], fp8, isOutput=False)
    hh = nc.declare_dram_parameter("h", [P, oblocks], f32, isOutput=False)
    gg = nc.declare_dram_parameter("g", [P, N_SH], bf16, isOutput=False)
    out = nc.declare_dram_parameter("out", [D_OUT, N_SH], bf16, isOutput=True)

    with tile.TileContext(nc) as tc, ExitStack() as ctx:
        consts = ctx.enter_context(tc.tile_pool(name="consts", bufs=1))
        bpool = ctx.enter_context(tc.tile_pool(name="bpool", bufs=1))
        apool = ctx.enter_context(tc.tile_pool(name="apool", bufs=2))
        outp = ctx.enter_context(tc.tile_pool(name="outp", bufs=3))
        outg = ctx.enter_context(tc.tile_pool(name="outg", bufs=3))
        # Per-ntile PSUM pools: 512-f32 tiles are exactly one 2KB bank, so
        # ntiles 0/1 run triple-buffered and ntile 2 double-buffered
        # (3+3+2 = 8 banks). The extra buffer gives the early blocks a full
        # block of recycle slack, riding out the scheduler's ~10-matmul-late
        # evac waits without stalling the in-order PE.
        psA = ctx.enter_context(tc.tile_pool(name="psA", bufs=3, space="PSUM"))
        psB = ctx.enter_context(tc.tile_pool(name="psB", bufs=3, space="PSUM"))
        psC = ctx.enter_context(tc.tile_pool(name="psC", bufs=2, space="PSUM"))
        NT_POOL = (psA, psB, psC)

        # B (11 KB/partition) and A (16 KB/partition double-buffered) fit in
        # SBUF. The prologue is HBM-bandwidth bound (~300 GB/s aggregate),
        # so it is scheduled by first need: B plane-pair halves alternate
        # between the sync and scalar queues ([0:512] halves first — they
        # feed ntile 0), the narrow A chunk 0 rides scalar, and later A
        # chunks stream on gpsimd. A-chunk tiles share a bufs=2 pool so
        # chunk q+2's DMA WAR-waits until chunk q is consumed — this paces
        # the A transfers and keeps them from starving the prologue/stores.
        bt = bpool.tile([P, KB, N_SH], fp8, name="bt")
        ats = {}

        def load_a_chunk(q):
            c0, w = CHUNKS[q]
            ats[q] = apool.tile([P, KB, 1024], fp8, tag="a", name=f"a_{q}")
            eng = nc.scalar if q == 0 else nc.gpsimd
            eng.dma_start(out=ats[q][:, :, 0:w], in_=at_d[:, :, c0:c0 + w])

        def shared_const(name, param, shape, dt):
            raw = consts.tile(shape, dt, name=f"{name}_raw")
            nc.scalar.dma_start(out=raw, in_=param[:, :])
            sb = consts.tile(shape, dt, name=f"{name}_sb")
            nc.vector.tensor_scalar(
                out=sb, in0=raw, scalar1=0.0, scalar2=None, op0=ALU.add,
            )
            return sb

        h_sb = shared_const("h", hh, [P, oblocks], f32)
        load_a_chunk(0)
        for half in ((0, BSPLIT), (BSPLIT, N_SH)):
            for d in range(KB // 2):
                eng = (nc.sync, nc.scalar)[d % 2]
                eng.dma_start(
                    out=bt[:, 2 * d:2 * d + 2, half[0]:half[1]],
                    in_=bt_d[:, 2 * d:2 * d + 2, half[0]:half[1]],
                )
        g_sb = shared_const("g", gg, [P, N_SH], bf16)
        # Pace chunk 1: DMA transfers fair-share the 16 engines, so issuing
        # c1 up front would steal ~1/3 of the prologue bandwidth from B.
        # A dummy DVE write into c1's tile that reads the last B half makes
        # the c1 DMA (WAW on the dummy) start only once B has landed.
        # Chunks 2+ are already WAR-paced by the bufs=2 pool recycle.
        ats[1] = apool.tile([P, KB, 1024], fp8, tag="a", name="a_1")
        nc.vector.tensor_scalar(
            out=ats[1][:, 0, 0:1], in0=bt[:, KB - 1, N_SH - 1:N_SH],
            scalar1=0.0, scalar2=None, op0=ALU.mult,
        )
        c0, w = CHUNKS[1]
        nc.gpsimd.dma_start(out=ats[1][:, :, 0:w], in_=at_d[:, :, c0:c0 + w])
        for q in range(2, len(CHUNKS)):
            load_a_chunk(q)

        # PE warm-up: dummy matmuls with no data deps run during the load
        # prologue so the HAM clock gate is hot when the real stream begins.
        # They write a psC-pool tile (no dedicated bank — psC recycles it).
        wl = consts.tile([P, 2, P], fp8, name="wl")
        nc.vector.memset(wl, 1.0)
        wr = consts.tile([P, 2, 512], fp8, name="wr")
        nc.vector.memset(wr, 1.0)
        pwt = psC.tile([P, 352], f32, tag="ps2", name="pwt",
                       padded_shape=[P, 512])
        for _ in range(N_WARMUP):
            nc.tensor.matmul(
                pwt, lhsT=wl[:, :, :], rhs=wr[:, :, 0:352],
                start=True, stop=True, perf_mode=DR,
            )

        def chunk_of(j):
            for q, (c0, w) in enumerate(CHUNKS):
                if c0 <= j * P < c0 + w:
                    return q, (j * P - c0)
            raise AssertionError(j)

        assert len(CHUNKS) - 2 <= 3  # WAR pacing depth with bufs=2

        for j in range(oblocks):
            q, coff = chunk_of(j)
            pts = [
                NT_POOL[n].tile(
                    [P, nw], f32, tag=f"ps{n}", name=f"ps_{j}_{n}",
                    padded_shape=[P, 512],
                )
                for n, (c0, nw) in enumerate(NTILES)
            ]

            def mm(n, d):
                c0, nw = NTILES[n]
                nc.tensor.matmul(
                    pts[n][:, 0:nw],
                    lhsT=ats[q][:, 2 * d:2 * d + 2, coff:coff + P],
                    rhs=bt[:, 2 * d:2 * d + 2, c0:c0 + nw],
                    start=(d == 0), stop=(d == KB // 2 - 1),
                    perf_mode=DR,
                )

            def evac_piece(n):
                c0, nw = NTILES[n]
                ot = outp.tile([P, nw], bf16, tag=f"out{n}", name=f"ot_{j}_{n}")
                og = outg.tile([P, nw], bf16, tag=f"og{n}", name=f"og_{j}_{n}")
                # blocks 0-1 evacuate on DVE straight from PSUM: the ACT
                # path's scheduler-assigned wait lands ~10 matmuls late for
                # the first blocks
                if j < 2:
                    nc.vector.tensor_scalar(
                        out=ot, in0=pts[n][:, 0:nw],
                        scalar1=h_sb[:, j:j + 1], scalar2=None, op0=ALU.mult,
                    )
                else:
                    nc.scalar.activation(
                        out=ot, in_=pts[n][:, 0:nw], func=AF.Copy,
                        scale=h_sb[:, j:j + 1],
                    )
                nc.vector.tensor_tensor(
                    out=og, in0=ot, in1=g_sb[:, c0:c0 + nw], op=ALU.mult,
                )
                # stores round-robin over the two HWDGE queues (DVE can't
                # issue DMAs); gpsimd keeps draining A-chunk loads
                eng = (nc.sync, nc.scalar)[(j + n) % 2]
                eng.dma_start(
                    out=out[j * P:(j + 1) * P, c0:c0 + nw], in_=og,
                )

            # Blocks 0, 2, 3 and the last run ntile-outer: block 0's first
            # matmuls then only need the [0:512] B halves (with no-dep dummy
            # matmuls between groups to keep the PE clock ramping through
            # the fill), blocks 2-3 delay their psC/pool-reuse writes past
            # the early evacs, and the last block's pieces evacuate inline
            # so the post-stream tail is one 352-col piece. ntile 2 (the
            # bufs=2 pool) evacuates first on dstep-outer blocks.
            if j in (0, 2, 3, oblocks - 1):
                for n in range(len(NTILES)):
                    for d in range(KB // 2):
                        mm(n, d)
                    evac_piece(n)
                    if j == 0 and n < 2:
                        for _ in range(3):
                            nc.tensor.matmul(
                                pwt, lhsT=wl[:, :, :], rhs=wr[:, :, 0:352],
                                start=True, stop=True, perf_mode=DR,
                            )
            else:
                for d in range(KB // 2):
                    for n in range(len(NTILES)):
                        mm(n, d)
                for n in (2, 0, 1):
                    evac_piece(n)

    nc.compile()
    return nc


_NC_CACHE = {}


def _get_nc():
    if "nc" not in _NC_CACHE:
        _NC_CACHE["nc"] = build_program()
    return _NC_CACHE["nc"]


def _pair_split_ell(ell):
    """Split each |ell_r| into a product a_r * b_r of e4m3 grid values.

    Returns (a, b_signed) as float32; a > 0, sign(ell) folded into b.
    """
    import ml_dtypes

    f8 = ml_dtypes.float8_e4m3
    grid = np.arange(256, dtype=np.uint8).view(f8).astype(np.float64)
    pos = np.unique(grid[np.isfinite(grid) & (grid > 0)])  # 119 values

    t = np.abs(ell).astype(np.float64)                     # (R,)
    q = t[:, None] / pos[None, :]                          # (R, 119)
    b = np.asarray(q, dtype=np.float64).astype(f8).astype(np.float64)
    bad = ~np.isfinite(b)
    prod = pos[None, :] * np.where(bad, 0.0, b)
    err = np.abs(prod - t[:, None])
    err[bad] = np.inf
    i = np.argmin(err, axis=1)
    a = pos[i]
    bsel = b[np.arange(len(t)), i]
    return (
        a.astype(np.float32),
        (bsel * np.where(ell >= 0, 1.0, -1.0)).astype(np.float32),
    )


def _make_in_maps(U_fp, V_fp, h, g, ell):
    U_fp = np.asarray(U_fp, dtype=np.float32)
    V_fp = np.asarray(V_fp, dtype=np.float32)
    h = np.asarray(h, dtype=np.float32).reshape(-1)
    g = np.asarray(g, dtype=np.float32).reshape(-1)
    ell = np.asarray(ell, dtype=np.float32).reshape(-1)

    import ml_dtypes

    bf = ml_dtypes.bfloat16
    f8 = ml_dtypes.float8_e4m3

    a, b = _pair_split_ell(ell)
    # A = sign(U^T) * a_r, B = sign(V^T) * b_r: +-a_r / +-b_r are e4m3 grid
    # values, so the fp8 casts are exact. Device layout is partition-major
    # [128, KB, cols]: row r = k*128 + p lands at [p, k, :].
    sgn_u = np.where(U_fp >= 0, np.float32(1.0), np.float32(-1.0))
    at = (sgn_u.T * a[:, None]).astype(f8)                        # (R, D_OUT)
    at = np.ascontiguousarray(at.reshape(KB, P, D_OUT).transpose(1, 0, 2))
    sgn_v = np.where(V_fp >= 0, np.float32(1.0), np.float32(-1.0))
    bt_full = (sgn_v.T * b[:, None]).astype(f8)                   # (R, D_IN)
    bt_full = bt_full.reshape(KB, P, D_IN).transpose(1, 0, 2)     # (P, KB, D_IN)

    h_t = np.ascontiguousarray(h.reshape(D_OUT // P, P).T)        # (128, 32)

    in_maps = []
    for c in range(NCORES):
        sl = slice(c * N_SH, (c + 1) * N_SH)
        in_maps.append({
            "at": at,
            "bt": np.ascontiguousarray(bt_full[:, :, sl]),        # (P, KB, N_SH)
            "h": h_t,
            "g": np.ascontiguousarray(
                np.broadcast_to(g[sl].astype(bf).reshape(1, N_SH), (P, N_SH))
            ),
        })
    return in_maps


def run(U_fp, V_fp, h, g, ell, trace=False):
    """Run on 8 NeuronCores; returns (M, BassKernelResults)."""
    from concourse.bass_utils import run_bass_kernel_spmd

    nc = _get_nc()
    in_maps = _make_in_maps(U_fp, V_fp, h, g, ell)
    res = run_bass_kernel_spmd(nc, in_maps, list(range(NCORES)), trace=trace)
    M = np.concatenate(
        [np.asarray(res.results[c]["out"]).astype(np.float32) for c in range(NCORES)],
        axis=1,
    )
    return M, res


def kernel(U_fp, V_fp, h, g, ell):
    M, _ = run(U_fp, V_fp, h, g, ell, trace=False)
    return M


# revision 34
# speedup vs baseline: 1.1337x; 1.1337x over previous
"""Trainium2 Bass kernel for LittleBitLinear reconstruction (fp8 DoubleRow).

Computes M = (sign(U_fp) * ell) @ sign(V_fp)^T * g[None, :] * h[:, None]
for U_fp (4096, 1024), V_fp (11008, 1024) -> M (4096, 11008) fp32.

Strategy: shard d_in (rows of V_fp / columns of M) across 8 cores; U_fp, h,
ell replicated. Each core computes the full 4096 x 1376 column block.

The contraction-dim scale ell is split per-r into a product of two fp8-e4m3
grid values ell_r ~= a_r * b_r (pair-optimized on host over the e4m3 grid, a
few % the error of a single fp8 round). Host input prep quantizes the sign
operands straight to fp8 (exact: +-a_r / +-b_r are grid values):
  A[r, o] = sign(U^T)[r, o] * a_r          (fp8)
  B[r, i] = sign(V^T)[r, i] * b_r          (fp8; sign(ell) folded in b)
so the device runs a pure matmul stream at the measured fp8-DoubleRow
roofline (1 col/cycle @ 2.4 GHz with K=256 per pass, 157 TF/s/core; 73.4 us
of PE work per core):
  psum    = A-slice^T @ B                  (DoubleRow, fp32 PSUM)
  out     = bf16(psum * h_o) * bf16(g_i)   (evac + DVE mult, bf16 store)
Host upcasts bf16 -> fp32. Measured end-to-end absmax rel err ~7.8e-3.

Layout/schedule notes (from perfetto traces):
- A/B ship partition-major [128, KB, cols] so one DMA covers all 8 k-planes
  of a column range (1024 descriptors instead of 8 separate plane DMAs).
- The prologue is HBM-bandwidth bound (~300 GB/s/core aggregate, fair-shared
  across in-flight transfers), so B halves ([0:512] first — they feed
  ntile 0) alternate sync/scalar, the narrow A chunk 0 rides scalar, and
  later A chunks are paced: chunk 1 is gated behind a dummy DVE op that
  reads the last B half (WAW on its tile), chunks 2+ WAR-pace through the
  bufs=2 pool recycle. Unpaced, the A transfers starve B and stall the PE.
- Block 0 runs ntile-outer (its first matmuls need only B's [0:512] halves)
  with no-dep dummy matmuls between groups to keep the PE p-state ramping
  through the fill; blocks 1+ run dstep-outer.
- Blocks 0-1 evacuate on DVE tensor_scalar straight from PSUM: the tile
  scheduler orders waits by its cost-model sim (which models fp8-DR 2x
  faster than real HW), handing the early ACT evacs a ~10-matmul-late wait
  that stalls the block-2 PSUM recycle.
- The last block's evac/store splits in halves across both HWDGE queues to
  shrink the post-stream tail.
"""

import os
import sys

import numpy as np

for _p in ("/opt/trn_rl_repo",):
    if _p not in sys.path and os.path.isdir(_p):
        sys.path.insert(0, _p)

D_OUT, D_IN, R, NCORES = 4096, 11008, 1024, 8
N_SH = D_IN // NCORES  # 1376
P = 128
KB = R // P            # 8 k-planes
NTILES = [(0, 512), (512, 512), (1024, 352)]
BSPLIT = 512           # B plane-pair DMA halves: [0:512], [512:1376]
# A chunks (d_out col ranges): narrow first chunk so block 0 unblocks fast
CHUNKS = [(0, 256), (256, 768), (1024, 1024), (2048, 1024), (3072, 1024)]
N_WARMUP = 4


def build_program():
    """Build the per-core Bass program (SPMD: same program, different data)."""
    from contextlib import ExitStack

    import concourse.bass as bass  # noqa: F401
    import concourse.mybir as mybir
    import concourse.tile as tile
    from concourse import bacc

    f32 = mybir.dt.float32
    bf16 = mybir.dt.bfloat16
    fp8 = mybir.dt.float8e4
    AF = mybir.ActivationFunctionType
    ALU = mybir.AluOpType
    DR = mybir.MatmulPerfMode.DoubleRow

    oblocks = D_OUT // P   # 32

    nc = bacc.Bacc(None, target_bir_lowering=False)
    # partition-major 3D layouts so one DMA covers all 8 k-planes of a slice
    at_d = nc.declare_dram_parameter("at", [P, KB, D_OUT], fp8, isOutput=False)
    bt_d = nc.declare_dram_parameter("bt", [P, KB, N_SH], fp8, isOutput=False)
    hh = nc.declare_dram_parameter("h", [P, oblocks], f32, isOutput=False)
    gg = nc.declare_dram_parameter("g", [P, N_SH], bf16, isOutput=False)
    out = nc.declare_dram_parameter("out", [D_OUT, N_SH], bf16, isOutput=True)

    with tile.TileContext(nc) as tc, ExitStack() as ctx:
        consts = ctx.enter_context(tc.tile_pool(name="consts", bufs=1))
        bpool = ctx.enter_context(tc.tile_pool(name="bpool", bufs=1))
        apool = ctx.enter_context(tc.tile_pool(name="apool", bufs=2))
        outp = ctx.enter_context(tc.tile_pool(name="outp", bufs=3))
        outg = ctx.enter_context(tc.tile_pool(name="outg", bufs=3))
        psum = ctx.enter_context(tc.tile_pool(name="psum", bufs=2, space="PSUM"))

        # B (11 KB/partition) and A (16 KB/partition double-buffered) fit in
        # SBUF. The prologue is HBM-bandwidth bound (~300 GB/s aggregate),
        # so it is scheduled by first need: B plane-pair halves alternate
        # between the sync and scalar queues ([0:512] halves first — they
        # feed ntile 0), the narrow A chunk 0 rides scalar, and later A
        # chunks stream on gpsimd. A-chunk tiles share a bufs=2 pool so
        # chunk q+2's DMA WAR-waits until chunk q is consumed — this paces
        # the A transfers and keeps them from starving the prologue/stores.
        bt = bpool.tile([P, KB, N_SH], fp8, name="bt")
        ats = {}

        def load_a_chunk(q):
            c0, w = CHUNKS[q]
            ats[q] = apool.tile([P, KB, 1024], fp8, tag="a", name=f"a_{q}")
            eng = nc.scalar if q == 0 else nc.gpsimd
            eng.dma_start(out=ats[q][:, :, 0:w], in_=at_d[:, :, c0:c0 + w])

        def shared_const(name, param, shape, dt):
            raw = consts.tile(shape, dt, name=f"{name}_raw")
            nc.scalar.dma_start(out=raw, in_=param[:, :])
            sb = consts.tile(shape, dt, name=f"{name}_sb")
            nc.vector.tensor_scalar(
                out=sb, in0=raw, scalar1=0.0, scalar2=None, op0=ALU.add,
            )
            return sb

        h_sb = shared_const("h", hh, [P, oblocks], f32)
        load_a_chunk(0)
        for half in ((0, BSPLIT), (BSPLIT, N_SH)):
            for d in range(KB // 2):
                eng = (nc.sync, nc.scalar)[d % 2]
                eng.dma_start(
                    out=bt[:, 2 * d:2 * d + 2, half[0]:half[1]],
                    in_=bt_d[:, 2 * d:2 * d + 2, half[0]:half[1]],
                )
        g_sb = shared_const("g", gg, [P, N_SH], bf16)
        # Pace chunk 1: DMA transfers fair-share the 16 engines, so issuing
        # c1 up front would steal ~1/3 of the prologue bandwidth from B.
        # A dummy DVE write into c1's tile that reads the last B half makes
        # the c1 DMA (WAW on the dummy) start only once B has landed.
        # Chunks 2+ are already WAR-paced by the bufs=2 pool recycle.
        ats[1] = apool.tile([P, KB, 1024], fp8, tag="a", name="a_1")
        nc.vector.tensor_scalar(
            out=ats[1][:, 0, 0:1], in0=bt[:, KB - 1, N_SH - 1:N_SH],
            scalar1=0.0, scalar2=None, op0=ALU.mult,
        )
        c0, w = CHUNKS[1]
        nc.gpsimd.dma_start(out=ats[1][:, :, 0:w], in_=at_d[:, :, c0:c0 + w])
        for q in range(2, len(CHUNKS)):
            load_a_chunk(q)

        # PE warm-up: dummy matmuls with no data deps run during the load
        # prologue so the HAM clock gate is hot when the real stream begins.
        wl = consts.tile([P, 2, P], fp8, name="wl")
        nc.vector.memset(wl, 1.0)
        wr = consts.tile([P, 2, 512], fp8, name="wr")
        nc.vector.memset(wr, 1.0)
        pw = ctx.enter_context(tc.tile_pool(name="psumw", bufs=1, space="PSUM"))
        pwt = pw.tile([P, 512], f32, name="pwt")
        for _ in range(N_WARMUP):
            nc.tensor.matmul(
                pwt, lhsT=wl[:, :, :], rhs=wr[:, :, :],
                start=True, stop=True, perf_mode=DR,
            )

        def chunk_of(j):
            for q, (c0, w) in enumerate(CHUNKS):
                if c0 <= j * P < c0 + w:
                    return q, (j * P - c0)
            raise AssertionError(j)

        assert len(CHUNKS) - 2 <= 3  # WAR pacing depth with bufs=2

        for j in range(oblocks):
            q, coff = chunk_of(j)
            pt = psum.tile(
                [P, N_SH], f32, tag="ps", name=f"ps_{j}",
                padded_shape=[P, 1536],
            )
            # block 0 runs ntile-outer so its first matmuls only need the
            # [0:512] B halves; later blocks run dstep-outer as usual. A few
            # no-dep dummy matmuls between block 0's ntile groups keep the
            # PE clock ramping while the next B halves are still in flight.
            if j == 0:
                for n, (c0, nw) in enumerate(NTILES):
                    for d in range(KB // 2):
                        nc.tensor.matmul(
                            pt[:, c0:c0 + nw],
                            lhsT=ats[q][:, 2 * d:2 * d + 2, coff:coff + P],
                            rhs=bt[:, 2 * d:2 * d + 2, c0:c0 + nw],
                            start=(d == 0), stop=(d == KB // 2 - 1),
                            perf_mode=DR,
                        )
                    if n < 2:
                        for _ in range(3):
                            nc.tensor.matmul(
                                pwt, lhsT=wl[:, :, :], rhs=wr[:, :, :],
                                start=True, stop=True, perf_mode=DR,
                            )
            else:
                for d in range(KB // 2):
                    lhsT = ats[q][:, 2 * d:2 * d + 2, coff:coff + P]
                    for (c0, nw) in NTILES:
                        nc.tensor.matmul(
                            pt[:, c0:c0 + nw], lhsT=lhsT,
                            rhs=bt[:, 2 * d:2 * d + 2, c0:c0 + nw],
                            start=(d == 0), stop=(d == KB // 2 - 1),
                            perf_mode=DR,
                        )
            # split the last block's evac/store in half across both queues
            # so the post-stream tail shrinks. Blocks 0-1 evacuate on DVE
            # (tensor_scalar h-mult straight from PSUM): the ACT path's
            # scheduler-assigned wait lands ~10 matmuls late for the first
            # blocks, which stalled the block-2 PSUM recycle.
            halves = (
                [(0, N_SH)] if j < oblocks - 1 else [(0, 688), (688, 688)]
            )
            for hi, (hc0, hw) in enumerate(halves):
                ot = outp.tile([P, hw], bf16, tag=f"out{hw}", name=f"ot_{j}_{hi}")
                og = outg.tile([P, hw], bf16, tag=f"og{hw}", name=f"og_{j}_{hi}")
                if j < 2:
                    nc.vector.tensor_scalar(
                        out=ot, in0=pt[:, hc0:hc0 + hw],
                        scalar1=h_sb[:, j:j + 1], scalar2=None, op0=ALU.mult,
                    )
                else:
                    nc.scalar.activation(
                        out=ot, in_=pt[:, hc0:hc0 + hw], func=AF.Copy,
                        scale=h_sb[:, j:j + 1],
                    )
                nc.vector.tensor_tensor(
                    out=og, in0=ot, in1=g_sb[:, hc0:hc0 + hw], op=ALU.mult,
                )
                # stores round-robin over the two HWDGE queues (DVE can't
                # issue DMAs); gpsimd keeps draining A-chunk loads
                eng = (nc.sync, nc.scalar)[(j + hi) % 2]
                eng.dma_start(
                    out=out[j * P:(j + 1) * P, hc0:hc0 + hw], in_=og,
                )

    nc.compile()
    return nc


_NC_CACHE = {}


def _get_nc():
    if "nc" not in _NC_CACHE:
        _NC_CACHE["nc"] = build_program()
    return _NC_CACHE["nc"]


def _pair_split_ell(ell):
    """Split each |ell_r| into a product a_r * b_r of e4m3 grid values.

    Returns (a, b_signed) as float32; a > 0, sign(ell) folded into b.
    """
    import ml_dtypes

    f8 = ml_dtypes.float8_e4m3
    grid = np.arange(256, dtype=np.uint8).view(f8).astype(np.float64)
    pos = np.unique(grid[np.isfinite(grid) & (grid > 0)])  # 119 values

    t = np.abs(ell).astype(np.float64)                     # (R,)
    q = t[:, None] / pos[None, :]                          # (R, 119)
    b = np.asarray(q, dtype=np.float64).astype(f8).astype(np.float64)
    bad = ~np.isfinite(b)
    prod = pos[None, :] * np.where(bad, 0.0, b)
    err = np.abs(prod - t[:, None])
    err[bad] = np.inf
    i = np.argmin(err, axis=1)
    a = pos[i]
    bsel = b[np.arange(len(t)), i]
    return (
        a.astype(np.float32),
        (bsel * np.where(ell >= 0, 1.0, -1.0)).astype(np.float32),
    )


def _make_in_maps(U_fp, V_fp, h, g, ell):
    U_fp = np.asarray(U_fp, dtype=np.float32)
    V_fp = np.asarray(V_fp, dtype=np.float32)
    h = np.asarray(h, dtype=np.float32).reshape(-1)
    g = np.asarray(g, dtype=np.float32).reshape(-1)
    ell = np.asarray(ell, dtype=np.float32).reshape(-1)

    import ml_dtypes

    bf = ml_dtypes.bfloat16
    f8 = ml_dtypes.float8_e4m3

    a, b = _pair_split_ell(ell)
    # A = sign(U^T) * a_r, B = sign(V^T) * b_r: +-a_r / +-b_r are e4m3 grid
    # values, so the fp8 casts are exact. Device layout is partition-major
    # [128, KB, cols]: row r = k*128 + p lands at [p, k, :].
    sgn_u = np.where(U_fp >= 0, np.float32(1.0), np.float32(-1.0))
    at = (sgn_u.T * a[:, None]).astype(f8)                        # (R, D_OUT)
    at = np.ascontiguousarray(at.reshape(KB, P, D_OUT).transpose(1, 0, 2))
    sgn_v = np.where(V_fp >= 0, np.float32(1.0), np.float32(-1.0))
    bt_full = (sgn_v.T * b[:, None]).astype(f8)                   # (R, D_IN)
    bt_full = bt_full.reshape(KB, P, D_IN).transpose(1, 0, 2)     # (P, KB, D_IN)

    h_t = np.ascontiguousarray(h.reshape(D_OUT // P, P).T)        # (128, 32)

    in_maps = []
    for c in range(NCORES):
        sl = slice(c * N_SH, (c + 1) * N_SH)
        in_maps.append({
            "at": at,
            "bt": np.ascontiguousarray(bt_full[:, :, sl]),        # (P, KB, N_SH)
            "h": h_t,
            "g": np.ascontiguousarray(
                np.broadcast_to(g[sl].astype(bf).reshape(1, N_SH), (P, N_SH))
            ),
        })
    return in_maps


def run(U_fp, V_fp, h, g, ell, trace=False):
    """Run on 8 NeuronCores; returns (M, BassKernelResults)."""
    from concourse.bass_utils import run_bass_kernel_spmd

    nc = _get_nc()
    in_maps = _make_in_maps(U_fp, V_fp, h, g, ell)
    res = run_bass_kernel_spmd(nc, in_maps, list(range(NCORES)), trace=trace)
    M = np.concatenate(
        [np.asarray(res.results[c]["out"]).astype(np.float32) for c in range(NCORES)],
        axis=1,
    )
    return M, res


def kernel(U_fp, V_fp, h, g, ell):
    M, _ = run(U_fp, V_fp, h, g, ell, trace=False)
    return M
